# revision 45
# baseline (speedup 1.0000x reference)
"""AttentionMIL Trainium2 kernel (fp8 encoder, v2).

Math (per bag of 512 instances):
    emb    = relu(x @ w_enc + b_enc)            [512, 128]
    a      = tanh(emb @ w_att + b_att)          [512, 64]
    logits = a @ w_score (+ b_score, dropped: softmax shift-invariant)
    attn   = softmax(logits) within the bag
    bag    = sum_i attn[i] * emb[i]             [128]
    score  = bag @ w_cls + b_cls                [2]

Distribution: data-parallel over bags. 8 NeuronCores, 8 bags (4096
instances) per core, weights replicated, no cross-core communication.
Each core returns its 8 bags' scores transposed [2, 8]; host stacks.

v2 changes vs the bf16 baseline (50.8 us -> 40.6 us measured):
 - x and w_enc are fp8 e4m3 (final rel err ~7e-3 vs the 2e-2 gate).
   Halves HBM traffic: 4.2 MB/core; the x DMA now lands in ~13 us
   (vs 47 us busy before).
 - x arrives in 8 per-bag DMA pieces of 0.52 MB with 4 KB contiguous
   per partition line (the old layout produced 8592 1-KB descriptors
   that capped the HWDGE ring at ~180 GB/s; large contiguous
   descriptors run at near line rate). Weights ride in 3 blob DMAs on
   the scalar queue, w8 first (the old 8 small serial weight DMAs
   delayed the first encoder matmul by ~5 us).
 - Encoder matmuls use DoubleRowSwInterleave: lhsT [128, 2, 128] fp8
   host-interleaved, contracting K=256 per instruction, 4 MMs per bag.
   (Plain DoubleRow and tensor_tensor_reduce both kill the exec unit
   on this toolchain — probes P1/P5c; DRS still streams rhs at 1
   fp8/cycle, so it saves instructions, not cycles.)
 - Attention tail processed per bag PAIR with array-tiled concurrent
   matmuls: a^T for two bags lands in one PSUM bank via col-split
   (tile_position (0,0)/(0,64)); logits via zero-padded w_score
   [64, 32] into quadrants (0,0)/(64,32) so one Exp call (with
   accum_out producing the softmax denominator for free) covers both
   bags; tanh also covers both bags per call. Halves ACT time.
 - Normalization folded into the broadcast matmul: lhsT is a row of
   1/denom (instead of ones), so the e-broadcast is already attention
   weights and the DVE mul+reduce emits normalized bag embeddings.
   Classifier matmuls accumulate per pair; the epilogue is bias + DMA.
 - Last pair: head is half-split to pipeline PE/ACT on the end chain.
   (Tried and reverted as regressions: reductions on ACT accum_out,
   bag-granular last tails, unhalved last head, split first DMA.)
"""

import sys

sys.path.insert(0, "/opt/trn_rl_repo")

import numpy as np

N_INST = 32768
N_BAGS = 64
D_IN = 1024
D_EMB = 128
D_ATT = 64
N_CLS = 2

N_CORES = 8
BAGS_PER_CORE = N_BAGS // N_CORES          # 8
INST_PER_BAG = N_INST // N_BAGS            # 512
INST_PER_CORE = N_INST // N_CORES          # 4096
PAIRS = BAGS_PER_CORE // 2                 # 4
DC = 4                                     # double-chunks of K=256
USE_DOUBLEROW = True                       # via DoubleRowSwInterleave: plain
                                           # DoubleRow kills the exec unit on
                                           # this toolchain (probe P1), but the
                                           # SW-interleaved variant works (P1b)
                                           # and halves encoder PE time

_CACHE = {}


def _build():
    import concourse.bacc as bacc
    import concourse.mybir as mybir
    import concourse.tile as tile

    f32 = mybir.dt.float32
    f32r = mybir.dt.float32r
    bf16 = mybir.dt.bfloat16
    fp8 = mybir.dt.float8e4
    AF = mybir.ActivationFunctionType
    ALU = mybir.AluOpType
    DRS = mybir.MatmulPerfMode.DoubleRowSwInterleave

    nc = bacc.Bacc("TRN2", target_bir_lowering=False, debug=False,
                   enable_asserts=False, num_devices=N_CORES)

    # x packed on host: [bag, 128 part, 8 chunk * 512 inst] fp8,
    # partition = din % 128, chunk = din // 128; 4 KB contiguous per
    # partition per piece
    xt = nc.dram_tensor("xt", [BAGS_PER_CORE, 128, 8 * INST_PER_BAG], fp8,
                        kind="ExternalInput")
    # w_enc packed on host: [128 part, dc, ktile, emb] flattened and
    # DoubleRowSwInterleave-interleaved
    w8 = nc.dram_tensor("w8", [128, DC * 2 * D_EMB], fp8, kind="ExternalInput")
    # all small bf16 weights in one blob: watt | wspad | ones
    wbf = nc.dram_tensor("wbf", [128, 160], bf16, kind="ExternalInput")
    # all small f32 weights in one blob: benc | battp | wcls | bcls | zero
    wf32 = nc.dram_tensor("wf32", [128, 8], f32, kind="ExternalInput")
    out = nc.dram_tensor("out", [N_CLS, BAGS_PER_CORE], f32,
                         kind="ExternalOutput")

    with tile.TileContext(nc) as tc:
        with (
            tc.tile_pool(name="const", bufs=1) as const,
            tc.tile_pool(name="xp", bufs=BAGS_PER_CORE) as xp_pool,
            tc.tile_pool(name="embp", bufs=6) as embp,
            tc.tile_pool(name="work", bufs=2) as work,
            tc.tile_pool(name="ps_emb", bufs=3, space="PSUM") as ps_emb,
            tc.tile_pool(name="ps_a", bufs=1, space="PSUM") as ps_a,
            tc.tile_pool(name="ps_l", bufs=1, space="PSUM") as ps_l,
            tc.tile_pool(name="ps_bc", bufs=2, space="PSUM") as ps_bc,
            tc.tile_pool(name="ps_cls", bufs=1, space="PSUM") as ps_cls,
        ):
            # ---- replicated weights: w8 rides FIRST on the sync queue
            # (the first encoder matmul blocks on it, and the scalar
            # queue's auto-inserted ACT_TABLE_LOAD would delay it ~1.3us);
            # the small blobs go on the scalar queue ----
            w8_sb = const.tile([128, DC, 2, D_EMB], fp8)
            nc.sync.dma_start(
                out=w8_sb,
                in_=w8[:, :].rearrange("p (a b e) -> p a b e", a=DC, b=2))
            wbf_sb = const.tile([128, 160], bf16)
            nc.scalar.dma_start(out=wbf_sb, in_=wbf[:, :])
            wf32_sb = const.tile([128, 8], f32)
            nc.scalar.dma_start(out=wf32_sb, in_=wf32[:, :])

            watt_sb = wbf_sb[:, 0:64]
            wspad_sb = wbf_sb[:, 64:96]
            ones_sb = wbf_sb[:, 96:160]
            benc_sb = wf32_sb[:, 0:1]
            battp_sb = wf32_sb[:, 1:2]
            wcls_sb = wf32_sb[:, 2:4]
            bcls_sb = wf32_sb[0:N_CLS, 4:5]
            zerob_sb = wf32_sb[0:64, 5:6]
            zerob128_sb = wf32_sb[:, 5:6]

            # normalized bag embeddings, column per bag
            bag_all = const.tile([D_EMB, BAGS_PER_CORE], f32)

            # ---- x: one 0.52 MB DMA per bag (sync queue) ----
            xpieces = []
            for b in range(BAGS_PER_CORE):
                xp = xp_pool.tile([128, 8, INST_PER_BAG], fp8, tag="xp",
                                  name=f"xp{b}")
                nc.sync.dma_start(
                    out=xp,
                    in_=xt[b, :, :].rearrange("p (c i) -> p c i", c=8))
                xpieces.append(xp)

            def emit_enc(b):
                # emb^T for bag b
                pse = ps_emb.tile([D_EMB, INST_PER_BAG], f32, tag="emb",
                                  name=f"pse{b}")
                xp = xpieces[b]
                if USE_DOUBLEROW:
                    # DoubleRow K=256 per MM (weights host-interleaved)
                    for dc in range(DC):
                        nc.tensor.matmul(
                            pse[:, :], w8_sb[:, dc, :, :],
                            xp[:, 2 * dc:2 * dc + 2, :],
                            start=(dc == 0), stop=(dc == DC - 1),
                            perf_mode=DRS)
                else:
                    for c in range(8):
                        nc.tensor.matmul(
                            pse[:, :], w8_sb[:, c // 2, c % 2, :], xp[:, c, :],
                            start=(c == 0), stop=(c == 7))
                return pse

            def emit_relu(pse, b):
                e = embp.tile([D_EMB, INST_PER_BAG], bf16, tag="embT",
                              name=f"embT{b}")
                nc.scalar.activation(e, pse, AF.Relu, bias=benc_sb, scale=1.0)
                return e

            def emit_pair_head(embs, sl, e_t, den_col):
                """att+tanh+score+exp for both bags over instance slice sl.

                Writes exp(logits) rows into e_t (row 0 = even bag, row 32 =
                odd bag) and per-row sums into den_col [64, 1].
                """
                n = sl.stop - sl.start
                # a^T pair: col-split quadrants of one PSUM bank
                ps_a_t = ps_a.tile([128, INST_PER_BAG], f32, tag="a")
                nc.tensor.matmul(ps_a_t[0:64, :n], watt_sb[:, :], embs[0][:, sl],
                                 start=True, stop=True, tile_position=(0, 0))
                nc.tensor.matmul(ps_a_t[64:128, :n], watt_sb[:, :], embs[1][:, sl],
                                 start=True, stop=True, tile_position=(0, 64))
                aT = work.tile([128, INST_PER_BAG], bf16, tag="aT")
                nc.scalar.activation(aT[:, :n], ps_a_t[:, :n], AF.Tanh,
                                     bias=battp_sb, scale=1.0)
                # logits: zero-padded w_score into two disjoint PE quadrants;
                # row 0 = even bag logits, row 32 = odd bag, rest zeros
                ps_l_t = ps_l.tile([64, INST_PER_BAG], f32, tag="logit")
                nc.tensor.matmul(ps_l_t[0:32, :n], wspad_sb[0:64, :],
                                 aT[0:64, 0:n],
                                 start=True, stop=True, tile_position=(0, 0))
                nc.tensor.matmul(ps_l_t[32:64, :n], wspad_sb[64:128, :],
                                 aT[64:128, 0:n],
                                 start=True, stop=True, tile_position=(64, 32))
                # exp with free softmax denominator (rows 1-31/33-63 hold
                # exp(0)=1 from the zero padding; harmless, never read).
                # No max-shift: |logits| <= ||w_score||_1 ~ 6, exp is safe.
                nc.scalar.activation(e_t[:, sl], ps_l_t[:, :n], AF.Exp,
                                     bias=zerob_sb, scale=1.0,
                                     accum_out=den_col)

            def emit_pair_norm_tail(p, embs, e_t, den_col):
                """1/denom -> normalized broadcast -> bag embeddings."""
                rv = work.tile([64, 1], f32, tag="rv")
                nc.vector.reciprocal(rv, den_col)
                rvrow = work.tile([64, 64], bf16, tag="rvrow")
                nc.vector.tensor_scalar_mul(rvrow, ones_sb[0:64, :], rv)
                for j in range(2):
                    r0 = 32 * j
                    b = 2 * p + j
                    ps_bc_t = ps_bc.tile([D_EMB, INST_PER_BAG], f32, tag="bc",
                                         name=f"bc{p}_{j}")
                    nc.tensor.matmul(ps_bc_t[0:64, :], rvrow[r0:r0 + 1, :],
                                     e_t[r0:r0 + 1, :], start=True, stop=True,
                                     tile_position=(r0, 0))
                    nc.tensor.matmul(ps_bc_t[64:128, :], rvrow[r0:r0 + 1, :],
                                     e_t[r0:r0 + 1, :], start=True, stop=True,
                                     tile_position=(r0, 64))
                    # tensor_tensor_reduce would fuse these, but it kills the
                    # exec unit on this toolchain (probe P5c)
                    scratch = work.tile([D_EMB, INST_PER_BAG], bf16,
                                        tag="scratch", name=f"sc{p}_{j}")
                    nc.vector.tensor_mul(scratch, embs[j], ps_bc_t)
                    nc.vector.reduce_sum(bag_all[:, b:b + 1], scratch,
                                         axis=mybir.AxisListType.X)
                # classifier contribution for this pair, accumulated into the
                # shared [2, 8] PSUM tile so only bias+DMA remain at the end
                nc.tensor.matmul(ps_s[:, 2 * p:2 * p + 2], wcls_sb,
                                 bag_all[:, 2 * p:2 * p + 2],
                                 start=True, stop=True)

            def emit_tail(p, embs):
                e_t = work.tile([64, INST_PER_BAG], bf16, tag="e")
                den_col = work.tile([64, 1], f32, tag="den")
                emit_pair_head(embs, slice(0, INST_PER_BAG), e_t, den_col)
                emit_pair_norm_tail(p, embs, e_t, den_col)

            def emit_tail_halved(p, embs):
                # last pair: half-split the head so PE/ACT stages pipeline
                # against each other on the end-of-kernel serial chain
                H = INST_PER_BAG // 2
                e_t = work.tile([64, INST_PER_BAG], bf16, tag="e")
                den_h = work.tile([64, 2], f32, tag="den_h")
                for h in range(2):
                    emit_pair_head(embs, slice(h * H, (h + 1) * H), e_t,
                                   den_h[:, h:h + 1])
                den_col = work.tile([64, 1], f32, tag="den")
                nc.vector.tensor_add(den_col, den_h[:, 0:1], den_h[:, 1:2])
                emit_pair_norm_tail(p, embs, e_t, den_col)

            # per-pair classifier contributions accumulate here
            ps_s = ps_cls.tile([N_CLS, BAGS_PER_CORE], f32, tag="cls")

            # software pipeline: pair p's tail emitted after pair p+1's
            # first encoder so the in-order PE queue never head-of-line
            # blocks on the softmax chain
            prev = None
            for p in range(PAIRS):
                pse0 = emit_enc(2 * p)
                if prev is not None:
                    emit_tail(p - 1, prev)
                e0 = emit_relu(pse0, 2 * p)
                pse1 = emit_enc(2 * p + 1)
                e1 = emit_relu(pse1, 2 * p + 1)
                prev = [e0, e1]
            emit_tail_halved(PAIRS - 1, prev)

            # scores^T already in ps_s; just bias and store   [2, 8]
            scores = work.tile([N_CLS, BAGS_PER_CORE], f32, tag="scores")
            nc.scalar.activation(scores, ps_s, AF.Identity, bias=bcls_sb,
                                 scale=1.0)
            nc.sync.dma_start(out=out[:, :], in_=scores)

    nc.compile()
    return nc


def prep_in_maps(inputs):
    """Pack full-size inputs into the per-core input maps."""
    import ml_dtypes

    fp8 = ml_dtypes.float8_e4m3
    bf16 = ml_dtypes.bfloat16

    x = np.asarray(inputs["x"], dtype=np.float32)
    w_enc = np.asarray(inputs["w_enc"], dtype=np.float32)
    b_enc = np.asarray(inputs["b_enc"], dtype=np.float32)
    w_att = np.asarray(inputs["w_att"], dtype=np.float32)
    b_att = np.asarray(inputs["b_att"], dtype=np.float32)
    w_score = np.asarray(inputs["w_score"], dtype=np.float32)
    w_cls = np.asarray(inputs["w_cls"], dtype=np.float32)
    b_cls = np.asarray(inputs["b_cls"], dtype=np.float32)

    # w_enc [1024, 128] -> [p, dc, ktile, emb]: din = dc*256 + ktile*128 + p
    w_dr = (w_enc.reshape(DC, 2, 128, D_EMB).transpose(2, 0, 1, 3)
            .astype(fp8))                  # [p, dc, ktile, emb]
    if USE_DOUBLEROW:
        # DoubleRowSwInterleave HW layout per dc:
        #   w_hw[p, 2*(127-m) + i] = w_dr[p, i, m]
        w8 = np.zeros((128, DC, 2 * D_EMB), dtype=fp8)
        w8[:, :, 0::2] = w_dr[:, :, 0, ::-1]
        w8[:, :, 1::2] = w_dr[:, :, 1, ::-1]
        w8 = np.ascontiguousarray(w8.reshape(128, DC * 2 * D_EMB))
    else:
        w8 = np.ascontiguousarray(w_dr.reshape(128, DC * 2 * D_EMB))
    # bf16 blob: watt | wspad | ones
    wbf = np.zeros((128, 160), dtype=bf16)
    wbf[:, 0:64] = w_att.astype(bf16)
    wbf[0:64, 64] = w_score.astype(bf16)
    wbf[64:128, 64] = w_score.astype(bf16)
    wbf[:, 96:160] = np.ones((128, 64), dtype=bf16)
    # f32 blob: benc | battp | wcls | bcls | zero
    wf32 = np.zeros((128, 8), dtype=np.float32)
    wf32[:, 0] = b_enc
    wf32[:, 1] = np.concatenate([b_att, b_att])
    wf32[:, 2:4] = w_cls
    wf32[0:N_CLS, 4] = b_cls

    shared = {"w8": w8, "wbf": wbf, "wf32": wf32}
    in_maps = []
    for c in range(N_CORES):
        xs = x[c * INST_PER_CORE:(c + 1) * INST_PER_CORE]
        # [4096, 1024] -> T -> [chunk, p, bag, inst] -> [bag, p, chunk, inst]
        xtc = np.ascontiguousarray(
            xs.T.reshape(8, 128, BAGS_PER_CORE, INST_PER_BAG)
            .transpose(2, 1, 0, 3)
            .reshape(BAGS_PER_CORE, 128, 8 * INST_PER_BAG)).astype(fp8)
        in_maps.append({"xt": xtc, **shared})
    return in_maps


def _numpy_fallback(x, seg, w_enc, b_enc, w_att, b_att, w_score, b_score,
                    w_cls, b_cls):
    emb = np.maximum(x @ w_enc + b_enc, 0.0)
    a = np.tanh(emb @ w_att + b_att)
    logits = a @ w_score + b_score[0]
    out = np.zeros((N_BAGS, N_CLS), dtype=np.float32)
    for bag in range(N_BAGS):
        mask = seg == bag
        lg = logits[mask]
        e = np.exp(lg - lg.max())
        attn = e / e.sum()
        bag_emb = attn @ emb[mask]
        out[bag] = bag_emb @ w_cls + b_cls
    return out


def kernel(**inputs):
    from concourse.bass_utils import run_bass_kernel_spmd

    seg = np.asarray(inputs["seg"], dtype=np.int32)
    expected_seg = np.repeat(np.arange(N_BAGS, dtype=np.int32), INST_PER_BAG)
    if not np.array_equal(seg, expected_seg):
        # Layout differs from the balanced bags this kernel is built for.
        return _numpy_fallback(
            np.asarray(inputs["x"], dtype=np.float32), seg,
            *(np.asarray(inputs[k], dtype=np.float32) for k in
              ("w_enc", "b_enc", "w_att", "b_att", "w_score", "b_score",
               "w_cls", "b_cls")))

    if "nc" not in _CACHE:
        _CACHE["nc"] = _build()
    nc = _CACHE["nc"]

    in_maps = prep_in_maps(inputs)
    res = run_bass_kernel_spmd(nc, in_maps, core_ids=list(range(N_CORES)))
    return np.concatenate(
        [res.results[c]["out"].T for c in range(N_CORES)], axis=0)


# revision 47
# speedup vs baseline: 1.0204x; 1.0204x over previous
"""AttentionMIL Trainium2 kernel (fp8 encoder, v2).

Math (per bag of 512 instances):
    emb    = relu(x @ w_enc + b_enc)            [512, 128]
    a      = tanh(emb @ w_att + b_att)          [512, 64]
    logits = a @ w_score (+ b_score, dropped: softmax shift-invariant)
    attn   = softmax(logits) within the bag
    bag    = sum_i attn[i] * emb[i]             [128]
    score  = bag @ w_cls + b_cls                [2]

Distribution: data-parallel over bags. 8 NeuronCores, 8 bags (4096
instances) per core, weights replicated, no cross-core communication.
Each core returns its 8 bags' scores transposed [2, 8]; host stacks.

v2 changes vs the bf16 baseline (50.8 us -> 40.6 us measured):
 - x and w_enc are fp8 e4m3 (final rel err ~7e-3 vs the 2e-2 gate).
   Halves HBM traffic: 4.2 MB/core; the x DMA now lands in ~13 us
   (vs 47 us busy before).
 - x arrives in 8 per-bag DMA pieces of 0.52 MB with 4 KB contiguous
   per partition line (the old layout produced 8592 1-KB descriptors
   that capped the HWDGE ring at ~180 GB/s; large contiguous
   descriptors run at near line rate). Weights ride in 3 blob DMAs on
   the scalar queue, w8 first (the old 8 small serial weight DMAs
   delayed the first encoder matmul by ~5 us).
 - Encoder matmuls use DoubleRowSwInterleave: lhsT [128, 2, 128] fp8
   host-interleaved, contracting K=256 per instruction, 4 MMs per bag.
   (Plain DoubleRow and tensor_tensor_reduce both kill the exec unit
   on this toolchain — probes P1/P5c; DRS still streams rhs at 1
   fp8/cycle, so it saves instructions, not cycles.)
 - Attention tail processed per bag PAIR with array-tiled concurrent
   matmuls: a^T for two bags lands in one PSUM bank via col-split
   (tile_position (0,0)/(0,64)); logits via zero-padded w_score
   [64, 32] into quadrants (0,0)/(64,32) so one Exp call (with
   accum_out producing the softmax denominator for free) covers both
   bags; tanh also covers both bags per call. Halves ACT time.
 - Normalization folded into the broadcast matmul: lhsT is a row of
   1/denom (instead of ones), so the e-broadcast is already attention
   weights and the DVE mul+reduce emits normalized bag embeddings.
   Classifier matmuls accumulate per pair; the epilogue is bias + DMA.
 - Last pair: head is half-split to pipeline PE/ACT on the end chain.
   (Tried and reverted as regressions: reductions on ACT accum_out,
   bag-granular last tails, unhalved last head, split first DMA.)
"""

import sys

sys.path.insert(0, "/opt/trn_rl_repo")

import numpy as np

N_INST = 32768
N_BAGS = 64
D_IN = 1024
D_EMB = 128
D_ATT = 64
N_CLS = 2

N_CORES = 8
BAGS_PER_CORE = N_BAGS // N_CORES          # 8
INST_PER_BAG = N_INST // N_BAGS            # 512
INST_PER_CORE = N_INST // N_CORES          # 4096
PAIRS = BAGS_PER_CORE // 2                 # 4
DC = 4                                     # double-chunks of K=256
USE_DOUBLEROW = True                       # via DoubleRowSwInterleave: plain
                                           # DoubleRow kills the exec unit on
                                           # this toolchain (probe P1), but the
                                           # SW-interleaved variant works (P1b)
                                           # and halves encoder PE time

_CACHE = {}


def _build():
    import concourse.bacc as bacc
    import concourse.mybir as mybir
    import concourse.tile as tile

    f32 = mybir.dt.float32
    f32r = mybir.dt.float32r
    bf16 = mybir.dt.bfloat16
    fp8 = mybir.dt.float8e4
    AF = mybir.ActivationFunctionType
    ALU = mybir.AluOpType
    DRS = mybir.MatmulPerfMode.DoubleRowSwInterleave

    nc = bacc.Bacc("TRN2", target_bir_lowering=False, debug=False,
                   enable_asserts=False, num_devices=N_CORES)

    # x packed on host: [bag, 128 part, 8 chunk * 512 inst] fp8,
    # partition = din % 128, chunk = din // 128; 4 KB contiguous per
    # partition per piece
    xt = nc.dram_tensor("xt", [BAGS_PER_CORE, 128, 8 * INST_PER_BAG], fp8,
                        kind="ExternalInput")
    # w_enc packed on host: [128 part, dc, ktile, emb] flattened and
    # DoubleRowSwInterleave-interleaved
    w8 = nc.dram_tensor("w8", [128, DC * 2 * D_EMB], fp8, kind="ExternalInput")
    # all small bf16 weights in one blob: watt | wspad | ones
    wbf = nc.dram_tensor("wbf", [128, 160], bf16, kind="ExternalInput")
    # all small f32 weights in one blob: benc | battp | wcls | bcls | zero
    wf32 = nc.dram_tensor("wf32", [128, 8], f32, kind="ExternalInput")
    out = nc.dram_tensor("out", [N_CLS, BAGS_PER_CORE], f32,
                         kind="ExternalOutput")

    with tile.TileContext(nc) as tc:
        with (
            tc.tile_pool(name="const", bufs=1) as const,
            tc.tile_pool(name="xp", bufs=BAGS_PER_CORE) as xp_pool,
            tc.tile_pool(name="embp", bufs=6) as embp,
            tc.tile_pool(name="work", bufs=2) as work,
            tc.tile_pool(name="ps_emb", bufs=3, space="PSUM") as ps_emb,
            tc.tile_pool(name="ps_a", bufs=1, space="PSUM") as ps_a,
            tc.tile_pool(name="ps_l", bufs=1, space="PSUM") as ps_l,
            tc.tile_pool(name="ps_bc", bufs=2, space="PSUM") as ps_bc,
            tc.tile_pool(name="ps_cls", bufs=1, space="PSUM") as ps_cls,
        ):
            # ---- replicated weights: w8 rides FIRST on the sync queue
            # (the first encoder matmul blocks on it, and the scalar
            # queue's auto-inserted ACT_TABLE_LOAD would delay it ~1.3us);
            # the small blobs go on the scalar queue ----
            w8_sb = const.tile([128, DC, 2, D_EMB], fp8)
            nc.sync.dma_start(
                out=w8_sb,
                in_=w8[:, :].rearrange("p (a b e) -> p a b e", a=DC, b=2))
            wbf_sb = const.tile([128, 160], bf16)
            nc.scalar.dma_start(out=wbf_sb, in_=wbf[:, :])
            wf32_sb = const.tile([128, 8], f32)
            nc.scalar.dma_start(out=wf32_sb, in_=wf32[:, :])

            watt_sb = wbf_sb[:, 0:64]
            wspad_sb = wbf_sb[:, 64:96]
            ones_sb = wbf_sb[:, 96:160]
            benc_sb = wf32_sb[:, 0:1]
            battp_sb = wf32_sb[:, 1:2]
            wcls_sb = wf32_sb[:, 2:4]
            bcls_sb = wf32_sb[0:N_CLS, 4:5]
            zerob_sb = wf32_sb[0:64, 5:6]
            zerob128_sb = wf32_sb[:, 5:6]

            # normalized bag embeddings, column per bag
            bag_all = const.tile([D_EMB, BAGS_PER_CORE], f32)

            # ---- x: one 0.52 MB DMA per bag (sync queue) ----
            xpieces = []
            for b in range(BAGS_PER_CORE):
                xp = xp_pool.tile([128, 8, INST_PER_BAG], fp8, tag="xp",
                                  name=f"xp{b}")
                nc.sync.dma_start(
                    out=xp,
                    in_=xt[b, :, :].rearrange("p (c i) -> p c i", c=8))
                xpieces.append(xp)

            def emit_enc(b):
                # emb^T for bag b
                pse = ps_emb.tile([D_EMB, INST_PER_BAG], f32, tag="emb",
                                  name=f"pse{b}")
                xp = xpieces[b]
                if USE_DOUBLEROW:
                    # DoubleRow K=256 per MM (weights host-interleaved)
                    for dc in range(DC):
                        nc.tensor.matmul(
                            pse[:, :], w8_sb[:, dc, :, :],
                            xp[:, 2 * dc:2 * dc + 2, :],
                            start=(dc == 0), stop=(dc == DC - 1),
                            perf_mode=DRS)
                else:
                    for c in range(8):
                        nc.tensor.matmul(
                            pse[:, :], w8_sb[:, c // 2, c % 2, :], xp[:, c, :],
                            start=(c == 0), stop=(c == 7))
                return pse

            def emit_relu(pse, b):
                e = embp.tile([D_EMB, INST_PER_BAG], bf16, tag="embT",
                              name=f"embT{b}")
                nc.scalar.activation(e, pse, AF.Relu, bias=benc_sb, scale=1.0)
                return e

            def emit_pair_head(embs, sl, e_t, den_col):
                """att+tanh+score+exp for both bags over instance slice sl.

                Writes exp(logits) rows into e_t (row 0 = even bag, row 32 =
                odd bag) and per-row sums into den_col [64, 1].
                """
                n = sl.stop - sl.start
                # a^T pair: col-split quadrants of one PSUM bank
                ps_a_t = ps_a.tile([128, INST_PER_BAG], f32, tag="a")
                nc.tensor.matmul(ps_a_t[0:64, :n], watt_sb[:, :], embs[0][:, sl],
                                 start=True, stop=True, tile_position=(0, 0))
                nc.tensor.matmul(ps_a_t[64:128, :n], watt_sb[:, :], embs[1][:, sl],
                                 start=True, stop=True, tile_position=(0, 64))
                aT = work.tile([128, INST_PER_BAG], bf16, tag="aT")
                nc.scalar.activation(aT[:, :n], ps_a_t[:, :n], AF.Tanh,
                                     bias=battp_sb, scale=1.0)
                # logits: zero-padded w_score into two disjoint PE quadrants;
                # row 0 = even bag logits, row 32 = odd bag, rest zeros
                ps_l_t = ps_l.tile([64, INST_PER_BAG], f32, tag="logit")
                nc.tensor.matmul(ps_l_t[0:32, :n], wspad_sb[0:64, :],
                                 aT[0:64, 0:n],
                                 start=True, stop=True, tile_position=(0, 0))
                nc.tensor.matmul(ps_l_t[32:64, :n], wspad_sb[64:128, :],
                                 aT[64:128, 0:n],
                                 start=True, stop=True, tile_position=(64, 32))
                # exp with free softmax denominator (rows 1-31/33-63 hold
                # exp(0)=1 from the zero padding; harmless, never read).
                # No max-shift: |logits| <= ||w_score||_1 ~ 6, exp is safe.
                nc.scalar.activation(e_t[:, sl], ps_l_t[:, :n], AF.Exp,
                                     bias=zerob_sb, scale=1.0,
                                     accum_out=den_col)

            def emit_pair_norm_tail(p, embs, e_t, den_col):
                """1/denom -> normalized broadcast -> bag embeddings."""
                rv = work.tile([64, 1], f32, tag="rv")
                nc.vector.reciprocal(rv, den_col)
                rvrow = work.tile([64, 64], bf16, tag="rvrow")
                nc.vector.tensor_scalar_mul(rvrow, ones_sb[0:64, :], rv)
                for j in range(2):
                    r0 = 32 * j
                    b = 2 * p + j
                    ps_bc_t = ps_bc.tile([D_EMB, INST_PER_BAG], f32, tag="bc",
                                         name=f"bc{p}_{j}")
                    nc.tensor.matmul(ps_bc_t[0:64, :], rvrow[r0:r0 + 1, :],
                                     e_t[r0:r0 + 1, :], start=True, stop=True,
                                     tile_position=(r0, 0))
                    nc.tensor.matmul(ps_bc_t[64:128, :], rvrow[r0:r0 + 1, :],
                                     e_t[r0:r0 + 1, :], start=True, stop=True,
                                     tile_position=(r0, 64))
                    # tensor_tensor_reduce would fuse these, but it kills the
                    # exec unit on this toolchain (probe P5c)
                    scratch = work.tile([D_EMB, INST_PER_BAG], bf16,
                                        tag="scratch", name=f"sc{p}_{j}")
                    nc.vector.tensor_mul(scratch, embs[j], ps_bc_t)
                    nc.vector.reduce_sum(bag_all[:, b:b + 1], scratch,
                                         axis=mybir.AxisListType.X)
                # classifier contribution for this pair, accumulated into the
                # shared [2, 8] PSUM tile so only bias+DMA remain at the end
                nc.tensor.matmul(ps_s[:, 2 * p:2 * p + 2], wcls_sb,
                                 bag_all[:, 2 * p:2 * p + 2],
                                 start=True, stop=True)

            def emit_tail(p, embs):
                e_t = work.tile([64, INST_PER_BAG], bf16, tag="e")
                den_col = work.tile([64, 1], f32, tag="den")
                emit_pair_head(embs, slice(0, INST_PER_BAG), e_t, den_col)
                emit_pair_norm_tail(p, embs, e_t, den_col)

            def emit_tail_halved(p, embs):
                # last pair: half-split the head so PE/ACT stages pipeline
                # against each other on the end-of-kernel serial chain
                H = INST_PER_BAG // 2
                e_t = work.tile([64, INST_PER_BAG], bf16, tag="e")
                den_h = work.tile([64, 2], f32, tag="den_h")
                for h in range(2):
                    emit_pair_head(embs, slice(h * H, (h + 1) * H), e_t,
                                   den_h[:, h:h + 1])
                den_col = work.tile([64, 1], f32, tag="den")
                nc.vector.tensor_add(den_col, den_h[:, 0:1], den_h[:, 1:2])
                emit_pair_norm_tail(p, embs, e_t, den_col)

            # per-pair classifier contributions accumulate here
            ps_s = ps_cls.tile([N_CLS, BAGS_PER_CORE], f32, tag="cls")

            # software pipeline: pair p's tail emitted after pair p+1's
            # first encoder so the in-order PE queue never head-of-line
            # blocks on the softmax chain
            prev = None
            for p in range(PAIRS):
                pse0 = emit_enc(2 * p)
                if prev is not None:
                    emit_tail(p - 1, prev)
                e0 = emit_relu(pse0, 2 * p)
                pse1 = emit_enc(2 * p + 1)
                e1 = emit_relu(pse1, 2 * p + 1)
                prev = [e0, e1]
            emit_tail_halved(PAIRS - 1, prev)

            # scores^T already in ps_s; just bias and store   [2, 8]
            scores = work.tile([N_CLS, BAGS_PER_CORE], f32, tag="scores")
            nc.scalar.activation(scores, ps_s, AF.Identity, bias=bcls_sb,
                                 scale=1.0)
            nc.sync.dma_start(out=out[:, :], in_=scores)

    nc.compile()
    return nc


def prep_in_maps(inputs):
    """Pack full-size inputs into the per-core input maps."""
    import ml_dtypes

    fp8 = ml_dtypes.float8_e4m3
    bf16 = ml_dtypes.bfloat16

    x = np.asarray(inputs["x"], dtype=np.float32)
    w_enc = np.asarray(inputs["w_enc"], dtype=np.float32)
    b_enc = np.asarray(inputs["b_enc"], dtype=np.float32)
    w_att = np.asarray(inputs["w_att"], dtype=np.float32)
    b_att = np.asarray(inputs["b_att"], dtype=np.float32)
    w_score = np.asarray(inputs["w_score"], dtype=np.float32)
    w_cls = np.asarray(inputs["w_cls"], dtype=np.float32)
    b_cls = np.asarray(inputs["b_cls"], dtype=np.float32)

    # w_enc [1024, 128] -> [p, dc, ktile, emb]: din = dc*256 + ktile*128 + p
    w_dr = (w_enc.reshape(DC, 2, 128, D_EMB).transpose(2, 0, 1, 3)
            .astype(fp8))                  # [p, dc, ktile, emb]
    if USE_DOUBLEROW:
        # DoubleRowSwInterleave HW layout per dc:
        #   w_hw[p, 2*(127-m) + i] = w_dr[p, i, m]
        w8 = np.zeros((128, DC, 2 * D_EMB), dtype=fp8)
        w8[:, :, 0::2] = w_dr[:, :, 0, ::-1]
        w8[:, :, 1::2] = w_dr[:, :, 1, ::-1]
        w8 = np.ascontiguousarray(w8.reshape(128, DC * 2 * D_EMB))
    else:
        w8 = np.ascontiguousarray(w_dr.reshape(128, DC * 2 * D_EMB))
    # bf16 blob: watt | wspad | ones
    wbf = np.zeros((128, 160), dtype=bf16)
    wbf[:, 0:64] = w_att.astype(bf16)
    wbf[0:64, 64] = w_score.astype(bf16)
    wbf[64:128, 64] = w_score.astype(bf16)
    wbf[:, 96:160] = np.ones((128, 64), dtype=bf16)
    # f32 blob: benc | battp | wcls | bcls | zero
    wf32 = np.zeros((128, 8), dtype=np.float32)
    wf32[:, 0] = b_enc
    wf32[:, 1] = np.concatenate([b_att, b_att])
    wf32[:, 2:4] = w_cls
    wf32[0:N_CLS, 4] = b_cls

    shared = {"w8": w8, "wbf": wbf, "wf32": wf32}
    in_maps = []
    for c in range(N_CORES):
        xs = x[c * INST_PER_CORE:(c + 1) * INST_PER_CORE]
        # [4096, 1024] -> T -> [chunk, p, bag, inst] -> [bag, p, chunk, inst]
        xtc = np.ascontiguousarray(
            xs.T.reshape(8, 128, BAGS_PER_CORE, INST_PER_BAG)
            .transpose(2, 1, 0, 3)
            .reshape(BAGS_PER_CORE, 128, 8 * INST_PER_BAG)).astype(fp8)
        in_maps.append({"xt": xtc, **shared})
    return in_maps


def _numpy_fallback(x, seg, w_enc, b_enc, w_att, b_att, w_score, b_score,
                    w_cls, b_cls):
    emb = np.maximum(x @ w_enc + b_enc, 0.0)
    a = np.tanh(emb @ w_att + b_att)
    logits = a @ w_score + b_score[0]
    out = np.zeros((N_BAGS, N_CLS), dtype=np.float32)
    for bag in range(N_BAGS):
        mask = seg == bag
        lg = logits[mask]
        e = np.exp(lg - lg.max())
        attn = e / e.sum()
        bag_emb = attn @ emb[mask]
        out[bag] = bag_emb @ w_cls + b_cls
    return out


def kernel(**inputs):
    from concourse.bass_utils import run_bass_kernel_spmd

    seg = np.asarray(inputs["seg"], dtype=np.int32)
    expected_seg = np.repeat(np.arange(N_BAGS, dtype=np.int32), INST_PER_BAG)
    if not np.array_equal(seg, expected_seg):
        # Layout differs from the balanced bags this kernel is built for.
        return _numpy_fallback(
            np.asarray(inputs["x"], dtype=np.float32), seg,
            *(np.asarray(inputs[k], dtype=np.float32) for k in
              ("w_enc", "b_enc", "w_att", "b_att", "w_score", "b_score",
               "w_cls", "b_cls")))

    if "nc" not in _CACHE:
        _CACHE["nc"] = _build()
    nc = _CACHE["nc"]

    in_maps = prep_in_maps(inputs)
    res = run_bass_kernel_spmd(nc, in_maps, core_ids=list(range(N_CORES)))
    return np.concatenate(
        [res.results[c]["out"].T for c in range(N_CORES)], axis=0)
